# revision 20
# baseline (speedup 1.0000x reference)
"""EntityAttentionLayer on 8 Trainium2 NeuronCores.

Data-parallel over batch (16 batches/core). The q/k projections run as fp8
(e4m3) DoubleRow matmuls: K=256 of contraction per PE instruction at the
same 216ns issue rate as a bf16 K=128 matmul, i.e. 2x. Host-side
quantization scales (entities x8, W_qk x256) are folded into the softmax
exp scale (2^-25). The v projection, logits, attention and output
projection stay bf16: fp8 error on those paths lands directly in the
output, while q/k error is attenuated through the softmax (logits are
small), keeping total rel-err ~1.2e-2 vs the 2e-2 gate.

Pipeline: batch b's attention matmuls (gated on the Scalar exp chain) are
interleaved into batch b+1's projection stream so the PE queue always has
independent work and the HAM clock gate never sees an idle window. Engine
split, chosen from measured per-op rates (DVE bf16 TT 0.42us, Pool TT
1.17us, psum-copy 0.7us, exp 0.7us):
  Vector: kT + v(cn0,1) + qT psum->sbuf copies, num-mul r=0, recip, norm
  Scalar: exp chain, v(cn2,3) copies (after the exps), den copies, out-mask
  GpSimd: num-mul r=1, ones memset, output DMA
  Sync:   group entity/mask DMAs, attn 128x512 DMA-xbar transpose
Emission order within each engine queue is deadline-ordered to avoid
head-of-line blocking (num-muls are emitted between projection copies).

Math note: the reference computes
    w = softmax(logits masked with -inf); w[nan] = 0
    w = w * diff; w = w / (sum(w) + 1e-8)
which equals num / sum(num) for num = exp(logits) * valid * (diff + 1e-8)
up to an O(1e-8) perturbation; fully-masked rows come out exactly 0 via
the +1e-25 denominator epsilon, matching the reference's NaN->0 path.
"""

import numpy as np
import ml_dtypes

BS, NE, NQ = 128, 512, 128
DIN, EMB, ODIM = 512, 512, 512
H, HD = 8, 64
NCORES = 8
BPC = BS // NCORES          # batches per core
GRP = 4                     # batches per q-projection group
EC = DIN // 128             # contraction chunks (4)
BF16 = ml_dtypes.bfloat16
F8 = ml_dtypes.float8_e4m3fn
SE = 8.0                    # entities fp8 scale
SW = 256.0                  # W_in q/k fp8 scale
EXP_SCALE = 1.0 / (8.0 * (SE * SW) ** 2)   # = 2^-25 (8 = sqrt(HD))


def _build_nc():
    import concourse.bacc as bacc
    import concourse.mybir as mybir
    import concourse.tile as tile
    from concourse.masks import make_identity

    f32 = mybir.dt.float32
    bf16 = mybir.dt.bfloat16
    f8 = mybir.dt.float8e4
    DR = mybir.MatmulPerfMode.DoubleRow
    Exp = mybir.ActivationFunctionType.Exp

    nc = bacc.Bacc("TRN2", target_bir_lowering=False, debug=False,
                   num_devices=NCORES)

    e8_d = nc.dram_tensor("ents8T", [BPC, DIN, NE], f8, kind="ExternalInput")
    e16_d = nc.dram_tensor("ents16T", [BPC, DIN, NE], bf16,
                           kind="ExternalInput")
    mask_d = nc.dram_tensor("maskT", [BPC, NE, NQ], bf16, kind="ExternalInput")
    w8_d = nc.dram_tensor("w_qk8T", [DIN, 2 * EMB], f8, kind="ExternalInput")
    wv_d = nc.dram_tensor("w_v16T", [DIN, EMB], bf16, kind="ExternalInput")
    wout_d = nc.dram_tensor("w_outT", [EMB, ODIM], bf16, kind="ExternalInput")
    pm_d = nc.dram_tensor("pmT", [NQ, BPC], f32, kind="ExternalInput")
    out_d = nc.dram_tensor("out", [BPC, NQ, ODIM], f32, kind="ExternalOutput")

    with tile.TileContext(nc) as tc:
        with (
            tc.tile_pool(name="const", bufs=1) as cpool,
            tc.tile_pool(name="gwork", bufs=2) as gwork,
            tc.tile_pool(name="work", bufs=3) as work,
            tc.tile_pool(name="nums", bufs=8) as nums,
            tc.tile_pool(name="ps", bufs=2, space="PSUM") as ps,
            tc.tile_pool(name="psl", bufs=2, space="PSUM") as psl_pool,
            tc.tile_pool(name="ps_att", bufs=2, space="PSUM") as ps_att,
        ):
            # ---- constants; batch 0's dependency chain (k weights,
            # ents8[0], wv) is spread across the three DMA queues and
            # issued before anything else ----
            w8_sb = cpool.tile([128, EC, 2 * EMB], f8)
            w8_r = w8_d.ap().rearrange("(c p) f -> p c f", p=128)
            for h in range(2):
                nc.sync.dma_start(
                    out=w8_sb[:, :, EMB + 256 * h:EMB + 256 * (h + 1)],
                    in_=w8_r[:, :, EMB + 256 * h:EMB + 256 * (h + 1)])
            wv_sb = cpool.tile([128, EC, EMB], bf16)
            wout_sb = cpool.tile([128, EC, ODIM], bf16)
            pm_sb = cpool.tile([128, BPC], f32)
            eps_sb = cpool.tile([128, 1], f32)
            ident = cpool.tile([128, 128], bf16)
            make_identity(nc, ident)
            nc.gpsimd.memset(eps_sb, 1e-25)

            # warm-up matmuls: keep the PE HAM at full clock while the
            # first weight and entity DMAs are in flight
            psum_w = ps.tile([128, 128], f32, tag="big", name="psum_w")
            for _ in range(35):
                nc.tensor.matmul(psum_w, lhsT=ident, rhs=ident,
                                 start=True, stop=True)

            def load_group0():
                """Group 0, per-batch DMAs ordered for batch-0 latency."""
                e8_sb = gwork.tile([128, GRP, EC, NE], f8, name="e8_sb")
                e16_sb = gwork.tile([128, GRP, EC, NE], bf16, name="e16_sb")
                eq8_sb = gwork.tile([128, EC, GRP, NQ], f8, name="eq8_sb")
                mask_sb = gwork.tile([128, GRP, EC * NQ], bf16, name="mask_sb")
                e8_r = [e8_d.ap()[b].rearrange("(c p) n -> p c n", p=128)
                        for b in range(GRP)]
                e16_r = [e16_d.ap()[b].rearrange("(c p) n -> p c n", p=128)
                         for b in range(GRP)]
                nc.sync.dma_start(out=e8_sb[:, 0, :, :], in_=e8_r[0])
                nc.gpsimd.dma_start(out=eq8_sb[:, :, 0, :],
                                    in_=e8_r[0][:, :, 0:NQ])
                nc.scalar.dma_start(
                    out=wv_sb, in_=wv_d.ap().rearrange("(c p) f -> p c f",
                                                       p=128))
                nc.gpsimd.dma_start(out=e16_sb[:, 0, :, :], in_=e16_r[0])
                nc.sync.dma_start(out=w8_sb[:, :, 0:EMB],
                                  in_=w8_r[:, :, 0:EMB])
                for i in range(1, GRP):
                    nc.sync.dma_start(out=e8_sb[:, i, :, :], in_=e8_r[i])
                    nc.gpsimd.dma_start(out=eq8_sb[:, :, i, :],
                                        in_=e8_r[i][:, :, 0:NQ])
                for i in range(GRP):
                    nc.gpsimd.dma_start(
                        out=mask_sb[:, i, :],
                        in_=mask_d.ap()[i].rearrange("(c p) q -> p c q",
                                                     p=128))
                    if i > 0:
                        nc.scalar.dma_start(out=e16_sb[:, i, :, :],
                                            in_=e16_r[i])
                nc.gpsimd.dma_start(
                    out=wout_sb,
                    in_=wout_d.ap().rearrange("(c p) f -> p c f", p=128))
                nc.gpsimd.dma_start(out=pm_sb, in_=pm_d.ap())
                return e8_sb, e16_sb, eq8_sb, mask_sb

            def load_group(g):
                """Prefetched groups: one strided DMA per tensor (sync)."""
                e8_sb = gwork.tile([128, GRP, EC, NE], f8, name="e8_sb")
                e16_sb = gwork.tile([128, GRP, EC, NE], bf16, name="e16_sb")
                eq8_sb = gwork.tile([128, EC, GRP, NQ], f8, name="eq8_sb")
                mask_sb = gwork.tile([128, GRP, EC * NQ], bf16, name="mask_sb")
                sl = slice(g * GRP, (g + 1) * GRP)
                nc.gpsimd.dma_start(
                    out=e8_sb,
                    in_=e8_d.ap()[sl].rearrange("i (c p) n -> p i c n", p=128))
                for i in range(GRP):
                    nc.gpsimd.dma_start(
                        out=eq8_sb[:, :, i, :],
                        in_=e8_d.ap()[g * GRP + i]
                            .rearrange("(c p) n -> p c n", p=128)[:, :, 0:NQ])
                nc.gpsimd.dma_start(
                    out=e16_sb,
                    in_=e16_d.ap()[sl].rearrange("i (c p) n -> p i c n",
                                                 p=128))
                nc.sync.dma_start(
                    out=mask_sb,
                    in_=mask_d.ap()[sl].rearrange("i (c p) q -> p i c q",
                                                  p=128))
                return e8_sb, e16_sb, eq8_sb, mask_sb

            def emit_logits_hc(i, hc, qT_sb, kT_sb):
                """Logits matmuls for one head pair into one 2-bank psl
                tile (row-group interleaved)."""
                pl = psl_pool.tile([128, 2, 4, NQ], f32, tag="psl",
                                   name="pl")
                for cn in range(4):
                    for r in range(2):
                        nc.tensor.matmul(
                            pl[:, r, cn, :],
                            lhsT=kT_sb[64 * r:64 * (r + 1), hc,
                                       128 * cn:128 * (cn + 1)],
                            rhs=qT_sb[64 * r:64 * (r + 1), hc, i, :],
                            start=True, stop=True)
                return pl

            def emit_exp(hc, pl):
                """one merged exp per head pair (Scalar)."""
                exp_sb = nums.tile([128, 8 * NQ], bf16, tag="exp",
                                   name="exp_sb")
                nc.scalar.activation(
                    exp_sb, pl.rearrange("p r c q -> p (r c q)"),
                    Exp, scale=EXP_SCALE)
                return exp_sb

            def emit_num(i, exp_sb, mask_sb):
                """one merged mask multiply per head pair (Vector); the
                mask broadcasts across the two row-group halves."""
                num_sb = nums.tile([128, 8 * NQ], bf16, tag="num",
                                   name="num_sb")
                nc.vector.tensor_mul(
                    num_sb.rearrange("p (r x) -> p r x", r=2),
                    exp_sb.rearrange("p (r x) -> p r x", r=2),
                    mask_sb[:, i, :].unsqueeze(1).broadcast_to(
                        (128, 2, EC * NQ)))
                return num_sb

            def qproj_chunks(grp_tiles):
                """fp8 DoubleRow qT projection for a whole group:
                qT[f, (i q)]; two PE chunks of 4 DR matmuls."""
                eq8_sb = grp_tiles[2]
                qT_sb = gwork.tile([128, 4, GRP, NQ], bf16, name="qT_sb")

                def chunk(cf0):
                    for cf in (cf0, cf0 + 1):
                        psum_q = ps.tile([128, GRP, NQ], f32, tag="big",
                                         name="psum_q")
                        for c in range(2):
                            nc.tensor.matmul(
                                psum_q,
                                lhsT=w8_sb[:, 2 * c:2 * c + 2,
                                           128 * cf:128 * (cf + 1)],
                                rhs=eq8_sb[:, 2 * c:2 * c + 2, :, :],
                                start=(c == 0), stop=(c == 1), perf_mode=DR)
                        if cf < 2:
                            nc.vector.tensor_copy(qT_sb[:, cf, :, :], psum_q)
                        else:
                            nc.scalar.copy(qT_sb[:, cf, :, :], psum_q)

                return qT_sb, [lambda: chunk(0), lambda: chunk(2)]

            def kT_chunks(i, grp_tiles):
                """fp8 DoubleRow kT projection for batch slot i: kT[f, n];
                two PE chunks of 4 DR matmuls + 2 Vector copies each."""
                e8_sb = grp_tiles[0]
                kT_sb = work.tile([128, 4, NE], bf16, name="kT_sb")

                def chunk(cf):
                    psum_k = ps.tile([128, NE], f32, tag="big",
                                     name="psum_k")
                    for c in range(2):
                        nc.tensor.matmul(
                            psum_k,
                            lhsT=w8_sb[:, 2 * c:2 * c + 2,
                                       EMB + 128 * cf:EMB + 128 * (cf + 1)],
                            rhs=e8_sb[:, i, 2 * c:2 * c + 2, :],
                            start=(c == 0), stop=(c == 1), perf_mode=DR)
                    nc.vector.tensor_copy(kT_sb[:, cf, :], psum_k)

                return kT_sb, [(lambda cf=cf: chunk(cf)) for cf in range(4)]

            def v_chunks(i, grp_tiles):
                """bf16 v projection (natural layout) + ones column; four PE
                chunks of 4 matmuls; copies cn0,1 on Vector, cn2,3 on Scalar
                (Scalar's land after the current batch's exp chain)."""
                e16_sb = grp_tiles[1]
                v_sb = work.tile([128, 4, H, HD + 1], bf16, name="v_sb")
                nc.gpsimd.memset(v_sb[:, :, :, HD], 1.0)

                def chunk(cn):
                    psum_v = ps.tile([128, EMB], f32, tag="big", name="psum_v")
                    for ce in range(EC):
                        nc.tensor.matmul(
                            psum_v,
                            lhsT=e16_sb[:, i, ce, 128 * cn:128 * (cn + 1)],
                            rhs=wv_sb[:, ce, :],
                            start=(ce == 0), stop=(ce == EC - 1))
                    src = psum_v.rearrange("p (h d) -> p h d", h=H)
                    if cn != 2:
                        nc.vector.tensor_copy(v_sb[:, cn, :, 0:HD], src)
                    else:
                        nc.scalar.copy(v_sb[:, cn, :, 0:HD], src)

                return v_sb, [(lambda cn=cn: chunk(cn)) for cn in range(4)]

            def emit_attn(hc, num_sb, v_sb, att_tiles):
                for r in range(2):
                    h = 2 * hc + r
                    patt, j = att_tiles[h // 4], h % 4
                    for cn in range(4):
                        o = 512 * r + 128 * cn
                        nc.tensor.matmul(
                            patt[:, j, :],
                            lhsT=num_sb[:, o:o + 128],
                            rhs=v_sb[:, cn, h, :],
                            start=(cn == 0), stop=(cn == 3))

            def tail_v(b, att_tiles):
                """denominators (S copies + V recip) + normalize (V) +
                transpose (Sync DMA xbar)."""
                dall_sb = work.tile([128, H], f32, name="dall_sb")
                nc.scalar.add(dall_sb[:, 0:4], att_tiles[0][:, :, HD], eps_sb)
                nc.scalar.add(dall_sb[:, 4:8], att_tiles[1][:, :, HD], eps_sb)
                recip_sb = work.tile([128, H], f32, name="recip_sb")
                nc.vector.reciprocal(recip_sb, dall_sb)

                attn_sb = work.tile([128, EMB], bf16, name="attn_sb")
                for t in range(2):
                    nc.vector.tensor_mul(
                        attn_sb[:, 256 * t:256 * (t + 1)]
                            .rearrange("p (h d) -> p h d", h=4),
                        att_tiles[t][:, :, 0:HD],
                        recip_sb[:, 4 * t:4 * t + 4].unsqueeze(2)
                            .broadcast_to((128, 4, HD)))
                attnT_sb = work.tile([128, 4, 128], bf16, name="attnT_sb")
                if b == BPC - 1:
                    nc.sync.dma_start_transpose(
                        attnT_sb[:, 0:2, :], attn_sb[:, 0:256])
                    nc.scalar.dma_start_transpose(
                        attnT_sb[:, 2:4, :], attn_sb[:, 256:512])
                else:
                    nc.sync.dma_start_transpose(attnT_sb, attn_sb)
                return attnT_sb

            def out_proj(b, attnT_sb):
                psum_o = ps.tile([128, ODIM], f32, tag="big", name="psum_o")
                for ct in range(4):
                    nc.tensor.matmul(
                        psum_o,
                        lhsT=attnT_sb[:, ct, :],
                        rhs=wout_sb[:, ct, :],
                        start=(ct == 0), stop=(ct == 3))
                out_sb = work.tile([128, ODIM], f32, name="out_sb")
                if b == BPC - 1:
                    # final batch: halve the fully-exposed tail chain
                    nc.scalar.mul(out_sb[:, 0:256], psum_o[:, 0:256],
                                  pm_sb[:, b:b + 1])
                    nc.vector.tensor_scalar_mul(
                        out_sb[:, 256:], psum_o[:, 256:], pm_sb[:, b:b + 1])
                    nc.gpsimd.dma_start(out=out_d.ap()[b, :, 0:256],
                                        in_=out_sb[:, 0:256])
                    nc.sync.dma_start(out=out_d.ap()[b, :, 256:],
                                      in_=out_sb[:, 256:])
                else:
                    nc.scalar.mul(out_sb, psum_o, pm_sb[:, b:b + 1])
                    nc.gpsimd.dma_start(out=out_d.ap()[b], in_=out_sb)

            # ---- software pipeline (one-period attention skew) ----
            # Period b emits: logits(b) + exp(b), the projection chunks for
            # b+1, the attention matmuls of b-1 (whose num tiles were
            # finished last period - so they never stall the PE), num(b),
            # tail_v(b-1), out_proj(b-2). A drain period finishes b=15.
            ngrp = BPC // GRP
            grp_tiles = [None] * ngrp
            qT = [None] * ngrp
            grp_tiles[0] = load_group0()

            # prologue: projections for batch 0
            kT_cur, kc = kT_chunks(0, grp_tiles[0])
            qT[0], qc = qproj_chunks(grp_tiles[0])
            v_prev = None
            v_cur, vc = v_chunks(0, grp_tiles[0])
            for c in [qc[0], kc[0], qc[1]] + kc[1:] + vc:
                c()

            nums_prev = None
            T1 = None             # attnT of b-2 at period start
            for b in range(BPC):
                g, i = divmod(b, GRP)
                mask_sb = grp_tiles[g][3]

                # projection chunks for b+1
                chunks = []
                if b + 1 < BPC:
                    if i == 1 and g + 1 < ngrp:
                        grp_tiles[g + 1] = load_group(g + 1)
                    g1, i1 = divmod(b + 1, GRP)
                    kT_next, kc = kT_chunks(i1, grp_tiles[g1])
                    v_next, vc = v_chunks(i1, grp_tiles[g1])
                    if i1 == 0:
                        qT[g1], qc = qproj_chunks(grp_tiles[g1])
                        chunks = [qc[0], kc[0], qc[1]] + kc[1:] + vc
                    else:
                        chunks = kc + vc
                else:
                    kT_next = v_next = None
                nci = 0

                def next_chunk():
                    nonlocal nci
                    if nci < len(chunks):
                        chunks[nci]()
                        nci += 1

                pls, exps, nums_cur = [], [], []
                for hc in range(4):
                    pls.append(emit_logits_hc(i, hc, qT[g], kT_cur))
                    exps.append(emit_exp(hc, pls[hc]))
                    next_chunk()
                if b > 0:
                    att_tiles = [
                        ps_att.tile([128, 4, HD + 1], f32, tag="att",
                                    name="pa0"),
                        ps_att.tile([128, 4, HD + 1], f32, tag="att",
                                    name="pa1"),
                    ]
                for hc in range(4):
                    if b > 0:
                        emit_attn(hc, nums_prev[hc], v_prev, att_tiles)
                    next_chunk()
                    nums_cur.append(emit_num(i, exps[hc], mask_sb))
                while nci < len(chunks):
                    next_chunk()

                if b > 0:
                    T0 = tail_v(b - 1, att_tiles)
                    if T1 is not None:
                        out_proj(b - 2, T1)
                    T1 = T0
                nums_prev = nums_cur
                kT_cur = kT_next
                v_prev, v_cur = v_cur, v_next

            # drain period: batch 15's attention + tails, with identity
            # filler matmuls so the HAM clock gate stays open through the
            # serial exp/normalize/transpose chain
            def filler(n):
                fill = ps.tile([128, 128], f32, tag="big", name="fill")
                for _ in range(n):
                    nc.tensor.matmul(fill, lhsT=ident, rhs=ident,
                                     start=True, stop=True)

            att_tiles = [
                ps_att.tile([128, 4, HD + 1], f32, tag="att", name="pa0"),
                ps_att.tile([128, 4, HD + 1], f32, tag="att", name="pa1"),
            ]
            for hc in range(4):
                emit_attn(hc, nums_prev[hc], v_prev, att_tiles)
            T0 = tail_v(BPC - 1, att_tiles)
            out_proj(BPC - 2, T1)
            filler(55)
            out_proj(BPC - 1, T0)

    nc.compile()
    return nc


def _prep_inputs(entities, pre_mask, diff_mask, post_mask, W_in, W_out):
    entities = np.asarray(entities, dtype=np.float32)
    pre_mask = np.asarray(pre_mask, dtype=bool)
    diff_mask = np.asarray(diff_mask, dtype=np.float32)
    post_mask = np.asarray(post_mask, dtype=bool)
    W_in = np.asarray(W_in, dtype=np.float32)
    W_out = np.asarray(W_out, dtype=np.float32)

    entsT = np.ascontiguousarray(entities.transpose(0, 2, 1))
    ents8T = (entsT * SE).astype(F8)
    ents16T = entsT.astype(BF16)
    m = (~pre_mask).astype(np.float32) * (diff_mask + 1e-8)
    maskT = np.ascontiguousarray(m.transpose(0, 2, 1)).astype(BF16)
    w_inT = np.ascontiguousarray(W_in.T)           # [DIN, 3E]
    w_qk8T = (w_inT[:, :2 * EMB] * SW).astype(F8)
    w_v16T = np.ascontiguousarray(w_inT[:, 2 * EMB:]).astype(BF16)
    w_outT = np.ascontiguousarray(W_out.T).astype(BF16)
    pmT = np.ascontiguousarray((~post_mask).T.astype(np.float32))

    in_maps = []
    for c in range(NCORES):
        sl = slice(c * BPC, (c + 1) * BPC)
        in_maps.append({
            "ents8T": np.ascontiguousarray(ents8T[sl]),
            "ents16T": np.ascontiguousarray(ents16T[sl]),
            "maskT": np.ascontiguousarray(maskT[sl]),
            "w_qk8T": w_qk8T,
            "w_v16T": w_v16T,
            "w_outT": w_outT,
            "pmT": np.ascontiguousarray(pmT[:, sl]),
        })
    return in_maps


def _run(in_maps, trace=False):
    from concourse.bass_utils import run_bass_kernel_spmd
    nc = _build_nc()
    last_exc = None
    for attempt in range(3):
        try:
            return run_bass_kernel_spmd(
                nc, in_maps, core_ids=list(range(NCORES)), trace=trace)
        except Exception as e:  # transient NRT_EXEC_UNIT faults on fresh NEFFs
            last_exc = e
            import time
            time.sleep(2.0 * (attempt + 1))
    raise last_exc


def kernel_traced(entities, pre_mask, diff_mask, post_mask, W_in, W_out, b_out,
                  trace=False):
    """Returns (output, BassKernelResults)."""
    b_out = np.asarray(b_out, dtype=np.float32)
    post_mask_np = np.asarray(post_mask, dtype=bool)
    in_maps = _prep_inputs(entities, pre_mask, diff_mask, post_mask, W_in, W_out)
    res = _run(in_maps, trace=trace)
    out = np.concatenate([r["out"] for r in res.results], axis=0)
    # faithfulness: reference adds b_out before the post-mask zeroing
    out = out + np.where(post_mask_np[:, :, None], 0.0, b_out[None, None, :])
    return out.astype(np.float32), res


def kernel(entities, pre_mask, diff_mask, post_mask, W_in, W_out, b_out):
    out, _ = kernel_traced(entities, pre_mask, diff_mask, post_mask,
                           W_in, W_out, b_out)
    return out
